# revision 4
# baseline (speedup 1.0000x reference)
"""Trainium2 Bass kernel for the masked per-site stencil contraction

    y[o, n] = f( sum_{i,k} Wconv[o,i,k] * mask[n,o,i,k] * x[i, shifts[n,k]] + bconv[o] )
    f(v) = (sigmoid(v) - 0.5) * (2 + 2e)/(e - 1) = (2+2e)/(2(e-1)) * tanh(v/2)

Shapes: O=I=32, K=13, N=4096.  Sharded over 8 NeuronCores along the site
dimension N (512 sites per core); mask / shifts / output columns are
partitioned, x / Wconv / bconv replicated.

Key idea vs the f32-mask version: the mask is binary, so it is bit-packed
host-side (32x smaller upload + HBM traffic) and expanded on chip:

  * maskp[p, og, jc, b] holds one bit per (site, o, i, k) packed along the
    site dim n (little-endian within each byte).  Partition layout matches
    the product tiles: p = kl*32 + i for chunk c (k = 4c + kl), jc = j*3+c
    for the 12 "big" k's, jc = 12 packs k=12 with p = j*32 + i.
  * expansion per tile: and8 = bitwise_and(bytes broadcast x8, pattern)
    (pattern[p, n] = 1 << (n%8)), then one fused DVE pass
    prod = (and8 != 0) * g  via scalar_tensor_tensor (exact: g is fp16).
  * gather g[p, n] = x[i(p), shifts[n, k(p)]] with GPSIMD ap_gather
    (x replicated to 128 partitions on chip; indices pre-wrapped host-side).
  * PE: per channel o a [128, 32] zero-padded-diagonal lhsT (only column o
    nonzero) accumulates W[o,:] . prod_o into row o of ONE [32, 512] PSUM
    tile; non-target rows accumulate exact zeros.  All 128 matmuls hit the
    same PSUM bank, so the tail is a single [32, 512] tanh + scale + DMA.
"""

import math

import numpy as np

import bass_rust
import concourse.bacc as bacc
import concourse.mybir as mybir
from concourse import tile
from concourse.bass_utils import run_bass_kernel_spmd

O, I, K, N = 32, 32, 13, 4096
NCORES = 8
NS = N // NCORES          # 512 local sites per core
IK = K * I                # 416 stencil rows, k-major: p = k*32 + i
BIG = 12 * I              # 384 rows in the three 128-partition chunks
_E = math.e
SCALE = (2.0 + 2.0 * _E) / (_E - 1.0)

_F32 = mybir.dt.float32
_F16 = mybir.dt.float16
_I16 = mybir.dt.int16
_U8 = mybir.dt.uint8

_BUILT = {}


def _diag_ap(base, pairs):
    """AP over `base`'s tensor: partition dim + explicit [stride, num] pairs."""
    return bass_rust.AP(base.tensor, 0, [list(base.ap[0])] + pairs)


def _emit(nc, tc, d, pools):
    cpool, gpool, apool, ppool, opool, qpool = pools
    AND = mybir.AluOpType.bitwise_and

    # --- constant loads (two HWDGE rings) -------------------------------
    maskp = cpool.tile([128, 8, 13, 64], _U8, tag="mp")
    nc.scalar.dma_start(
        maskp[:, :, :, :],
        d["maskp"][:, :].rearrange("p (og jc b) -> p og jc b", og=8, jc=13),
    )
    pat = cpool.tile([128, 64, 8], _U8, tag="pat")
    nc.scalar.dma_start(pat[:, :, :], d["pat"][:, :].rearrange("p (a b) -> p a b", b=8))
    x_sb = cpool.tile([128, N], _F32, tag="x")
    for r in range(4):
        eng = nc.sync if r < 2 else nc.scalar
        eng.dma_start(x_sb[32 * r : 32 * (r + 1), :], d["x"][:, :])
    idxb_sb = cpool.tile([128, 96], _I16, tag="ib")
    nc.sync.dma_start(idxb_sb[:, :], d["idxb"][:, :])
    idx3_sb = cpool.tile([128, 32], _I16, tag="i3")
    nc.sync.dma_start(idx3_sb[:, :], d["idx3"][:, :])
    wt_big = cpool.tile([128, 3, O], _F32, tag="wb")
    nc.sync.dma_start(
        wt_big[:, :, :], d["wt"][0:BIG, :].rearrange("(c p) m -> p c m", p=128)
    )
    wt3f_sb = cpool.tile([128, 4, O], _F32, tag="w3")
    nc.sync.dma_start(wt3f_sb[:, :, :], d["wt3f"][:, :, :].rearrange("j p m -> p j m"))
    bcol_sb = cpool.tile([O, 1], _F32, tag="bc")
    nc.sync.dma_start(bcol_sb[:, :], d["bcol"][:, :])

    # --- zero-padded diagonal weights (ACT engine, idle early) ----------
    # wz[p, c, o, m] = W-chunk-c[p, o] if m == o else 0; lhsT = wz[:, c, o, :]
    wz = cpool.tile([128, 3, O, O], _F16, tag="wz")
    nc.vector.memset(wz[:, :, :, :], 0.0)
    nc.scalar.copy(
        _diag_ap(wz[:, :, :, :], [[O * O, 3], [O + 1, O]]), wt_big[:, :, :]
    )
    # wz12[p, o, m]: column o holds wt3f's j(o)-block (p = j*32+i rows)
    wz12 = cpool.tile([128, O, O], _F16, tag="wz12")
    nc.vector.memset(wz12[:, :, :], 0.0)
    # src (og, j): wt3f_sb[p, j, 4og+j] -> flat 4*og + 33*j
    nc.scalar.copy(
        _diag_ap(wz12[:, :, :], [[4 * (O + 1), 8], [O + 1, 4]]),
        bass_rust.AP(
            wt3f_sb[:, :, :].tensor, 0,
            [list(wt3f_sb[:, :, :].ap[0]), [4, 8], [O + 1, 4]],
        ),
    )
    bhalf = cpool.tile([O, 1], _F32, tag="bh")
    nc.scalar.activation(
        bhalf[:, :], bcol_sb[:, :], mybir.ActivationFunctionType.Copy, scale=0.5
    )

    # --- gathers (GPSIMD), in PE consumption order: c=0,1,2 then k=12 ---
    g_big = gpool.tile([128, 3, NS], _F32, tag="g")
    for c in range(3):
        nc.gpsimd.ap_gather(
            g_big[:, c, :], x_sb[:, :], idxb_sb[:, 32 * c : 32 * c + 32],
            channels=128, num_elems=N, d=1, num_idxs=NS,
        )
    g3rep = gpool.tile([128, NS], _F32, tag="g3")
    nc.gpsimd.ap_gather(
        g3rep[:, :], x_sb[:, :], idx3_sb[:, :],
        channels=128, num_elems=N, d=1, num_idxs=NS,
    )

    # --- mask bit expansion: all ANDs up front (DVE busy while gathers run)
    and_tiles = {}
    for c in range(3):
        for og in range(8):
            a8 = apool.tile([128, 4, NS], _U8, tag=f"a{c}_{og}", bufs=1)
            src = maskp[:, og, 0:12, :].rearrange("p (j cc) b -> p j cc b", cc=3)
            nc.vector.tensor_tensor(
                a8[:, :, :].rearrange("p j (a b) -> p j a b", b=8),
                src[:, :, c, :].unsqueeze(3).broadcast_to([128, 4, 64, 8]),
                pat[:, :, :].unsqueeze(1).broadcast_to([128, 4, 64, 8]),
                op=AND,
            )
            and_tiles[(c, og)] = a8
    and12 = {}
    for og in range(8):
        a8 = apool.tile([128, NS], _U8, tag=f"a12_{og}", bufs=1)
        nc.vector.tensor_tensor(
            a8[:, :].rearrange("p (a b) -> p a b", b=8),
            maskp[:, og, 12, :].unsqueeze(2).broadcast_to([128, 64, 8]),
            pat[:, :, :],
            op=AND,
        )
        and12[og] = a8

    # --- products: prod = (and8 != 0) * g, one fused DVE pass per tile --
    gb = gpool.tile([128, 3, NS], _F16, tag="gb")
    prods = {}
    for c in range(3):
        nc.vector.tensor_copy(gb[:, c, :], g_big[:, c, :])
        for og in range(8):
            pr = ppool.tile([128, 4, NS], _F16, tag="pr", bufs=8)
            nc.vector.scalar_tensor_tensor(
                pr[:, :, :], and_tiles[(c, og)][:, :, :], 0.0,
                gb[:, c, :].unsqueeze(1).broadcast_to([128, 4, NS]),
                op0=mybir.AluOpType.not_equal, op1=mybir.AluOpType.mult,
            )
            prods[(c, og)] = pr
    g3b = gpool.tile([128, NS], _F16, tag="g3b")
    nc.vector.tensor_copy(g3b[:, :], g3rep[:, :])
    p3s = {}
    for og in range(8):
        p3 = ppool.tile([128, NS], _F16, tag=f"p3_{og}", bufs=1)
        nc.vector.scalar_tensor_tensor(
            p3[:, :], and12[og][:, :], 0.0, g3b[:, :],
            op0=mybir.AluOpType.not_equal, op1=mybir.AluOpType.mult,
        )
        p3s[og] = p3

    # --- PE: chunk-major passes, all channels into one [32, 512] bank ---
    yacc = qpool.tile([O, NS], _F32, tag="yacc")
    first = True
    for c in range(3):
        for og in range(8):
            for j in range(4):
                o = 4 * og + j
                nc.tensor.matmul(
                    yacc[:, :], wz[:, c, o, :], prods[(c, og)][:, j, :],
                    start=first, stop=False,
                )
                first = False
    for og in range(8):
        for j in range(4):
            o = 4 * og + j
            nc.tensor.matmul(
                yacc[:, :], wz12[:, o, :], p3s[og][:, :],
                start=False, stop=(og == 7 and j == 3),
            )

    # --- tail: y = SCALE/2 * tanh(0.5 v + 0.5 b) ------------------------
    ystage = opool.tile([O, NS], _F32, tag="ys")
    nc.scalar.activation(
        ystage[:, :], yacc[:, :], mybir.ActivationFunctionType.Tanh,
        bias=bhalf[:, 0:1], scale=0.5,
    )
    nc.vector.tensor_scalar_mul(ystage[:, :], ystage[:, :], SCALE / 2.0)
    nc.sync.dma_start(d["y"][:, :], ystage[:, :])


def _declare(nc):
    d = {}
    d["x"] = nc.declare_dram_parameter("x", [I, N], _F32, isOutput=False)
    d["wt"] = nc.declare_dram_parameter("wt", [IK, O], _F32, isOutput=False)
    d["wt3f"] = nc.declare_dram_parameter("wt3f", [4, 128, O], _F32, isOutput=False)
    d["bcol"] = nc.declare_dram_parameter("bcol", [O, 1], _F32, isOutput=False)
    d["maskp"] = nc.declare_dram_parameter("maskp", [128, 8 * 13 * 64], _U8, isOutput=False)
    d["pat"] = nc.declare_dram_parameter("pat", [128, 512], _U8, isOutput=False)
    d["idxb"] = nc.declare_dram_parameter("idxb", [128, 96], _I16, isOutput=False)
    d["idx3"] = nc.declare_dram_parameter("idx3", [128, 32], _I16, isOutput=False)
    d["y"] = nc.declare_dram_parameter("y", [O, NS], _F32, isOutput=True)
    return d


def _pools(tc, stack):
    names = [
        ("const", 1), ("gather", 1), ("and", 1), ("prod", 1),
        ("out", 1), ("psum", 1),
    ]
    pools = []
    for name, bufs in names:
        kw = {"space": "PSUM"} if name == "psum" else {}
        pools.append(stack.enter_context(tc.tile_pool(name=name, bufs=bufs, **kw)))
    return pools


def _build():
    """Build + compile the SPMD Bass program once per process."""
    if "nc" in _BUILT:
        return _BUILT["nc"]
    from contextlib import ExitStack

    nc = bacc.Bacc("TRN2", target_bir_lowering=False, debug=False)
    d = _declare(nc)
    with tile.TileContext(nc) as tc:
        with ExitStack() as stack:
            pools = _pools(tc, stack)
            _emit(nc, tc, d, pools)
    nc.compile()
    _BUILT["nc"] = nc
    return nc


def _wrap16(col):
    """shifts column (NS,) -> (16, NS//16) wrapped layout: out[r, s] = col[s*16+r]."""
    return np.ascontiguousarray(col.reshape(NS // 16, 16).T)


def make_in_maps(x, Wconv, bconv, mask, shifts):
    """Host-side shard/layout prep: pure data movement + bit-packing of the
    binary mask (values are exactly 0.0/1.0) + int32->int16 index narrowing
    (indices < 4096)."""
    x = np.ascontiguousarray(x, dtype=np.float32)                   # (32, N)
    W = Wconv.astype(np.float32, copy=False)
    wt = np.ascontiguousarray(W.transpose(2, 1, 0)).reshape(IK, O)  # (416, 32)
    # zero-padded k=12 weight columns: wt3f[j, 32j+i, o] = W[o, i, 12]
    wt3f = np.zeros((4, 128, O), np.float32)
    for j in range(4):
        wt3f[j, 32 * j : 32 * j + 32, :] = W[:, :, 12].T
    bcol = np.ascontiguousarray(
        bconv.astype(np.float32, copy=False).reshape(O, 1)
    )
    pat = np.ascontiguousarray(
        np.tile(np.array([1 << b for b in range(8)], np.uint8), 64)[None, :]
        .repeat(128, 0)
    )
    mask = np.asarray(mask)
    shifts = np.asarray(shifts)

    in_maps = []
    for core in range(NCORES):
        sl = slice(core * NS, (core + 1) * NS)
        mt = (mask[sl] != 0).transpose(1, 3, 2, 0)                  # (O, K, I, NS) bool
        m4 = mt.reshape(8, 4, 13, 32, NS)
        big = (
            m4[:, :, :12]
            .reshape(8, 4, 3, 4, 32, NS)                            # og j c kl i n
            .transpose(3, 4, 0, 1, 2, 5)                            # kl i og j c n
            .reshape(128, 8, 12, NS)
        )
        k12 = m4[:, :, 12].transpose(1, 2, 0, 3).reshape(128, 8, 1, NS)  # p=j*32+i
        bits = np.concatenate([big, k12], axis=2)                   # (128, 8, 13, NS)
        maskp = np.packbits(bits, axis=-1, bitorder="little").reshape(128, 8 * 13 * 64)
        sh = shifts[sl].astype(np.int16)                            # (NS, 13)
        idxb = np.empty((128, 96), np.int16)
        for g in range(8):
            for c in range(3):
                idxb[16 * g : 16 * g + 16, 32 * c : 32 * c + 32] = _wrap16(
                    sh[:, 4 * c + g // 2]
                )
        w12 = _wrap16(sh[:, 12])
        idx3 = np.empty((128, 32), np.int16)
        for g in range(8):
            idx3[16 * g : 16 * g + 16, :] = w12
        in_maps.append(
            {
                "x": x,
                "wt": wt,
                "wt3f": wt3f,
                "bcol": bcol,
                "maskp": maskp,
                "pat": pat,
                "idxb": idxb,
                "idx3": idx3,
            }
        )
    return in_maps


def kernel(x, Wconv, bconv, mask, shifts):
    nc = _build()
    in_maps = make_in_maps(x, Wconv, bconv, mask, shifts)
    res = run_bass_kernel_spmd(nc, in_maps, core_ids=list(range(NCORES)))
    y = np.empty((O, N), np.float32)
    for core in range(NCORES):
        y[:, core * NS : (core + 1) * NS] = res.results[core]["y"]
    return y
